# revision 42
# baseline (speedup 1.0000x reference)
"""Trainium2 Bass kernel for nn_FLinear2d (per-channel double linear).

Math (see reference):
  u[b,i,o] = sum_s U3[o,i,s] * x[b,i,s] + bU[o]        (64 per-channel matmuls)
  z[b,o,t] = sum_i V3[t,o,i] * u[b,i,o] + bV[t]        (128 per-o matmuls)

Two SPMD launches over 8 cores, all matmul operands in bf16 (fp32 PSUM
accumulation).  The problem is HBM-bound — U alone is 134 MB fp32 — and the
2e-2 gate leaves ~10x margin over bf16's ~3e-3 error (K=4096 accumulation in
fp32), so bf16 halves the dominant traffic and quadruples PE throughput.

  Stage A: shard C_in (8 channels/core).  Per (i, s-chunk):
      psum[o=128, b=64] += Uh[i,:,c,:].T @ Xh[i,:,c,:]   (bf16 in, fp32 acc)
    accumulated over 32 s-chunks -> u_base[o, i, b] (bf16 out).
  Stage B: shard C_out (16 o/core), pure K=64 contraction (biases are a
  host-side rank-1 table: bias[o,t] = bU[o]*sum_i V3[t,o,i] + bV[t], added
  in fp32 after unsharding).  V arrives o-PAIR packed: vh[p] = [128, S_OUT]
  with rows 0..63 = o=2p's V block and 64..127 = o=2p+1's, so every DMA
  uses all 128 SBUF partitions.  us is loaded once and duplicated on-chip
  (SBUF->SBUF) so both halves have a matching base partition.
    Per (o, t-tile): psum[t=128, b=64] = vt[half][:, tt*128:+128].T @ us[o]
    (single matmul, K=64), stored to HBM as bf16 pair blocks.

All DMAs are contiguous thanks to host-side layout transforms.
"""

import numpy as np
import ml_dtypes
from contextlib import ExitStack

import concourse.bass as bass
import concourse.tile as tile
from concourse import bacc, mybir
from concourse.bass_utils import run_bass_kernel_spmd

F32 = mybir.dt.float32
BF16 = mybir.dt.bfloat16
NP_BF16 = ml_dtypes.bfloat16
N_CORES = 8
CORE_IDS = list(range(N_CORES))

B, CI, CO = 64, 64, 128
S_IN, S_OUT = 4096, 1024
NCH = 32            # s-chunks of 128
I_PER_CORE = CI // N_CORES     # 8
O_PER_CORE = CO // N_CORES     # 16
TT = S_OUT // 128   # 8 t-tiles per o

_cache = {}


def _build_stage_a(repeat=1, dve_chunks=18):
    # U streams from HBM as int8 (it is uniformly distributed, so symmetric
    # int8 has ~0.4% RMS error vs fp8's 3.6% — measured end-to-end 5.5e-3
    # against the 2e-2 gate).  The dequant is a pure int8->bf16 cast
    # (integers <=127 are exact in bf16); the quant step is folded into x
    # on the host, so no extra scaling op exists anywhere on device.
    # Casts split DVE (0.58 ns/elem) / Pool (0.83); both rings carry
    # (U/2 + x/2) at ~12.6 us, all under the 23.8 us HBM floor.
    # (x-int8 as well was tried: floor 17.9us but the cast stream becomes
    # the wall at ~23.4us sim and error rises to 1.2e-2 — not worth it.)
    # HYBRID x: chunks 0..15 stream as int8 (cast on-chip), 16..31 as bf16
    # pre-scaled by 1/sx on the host, so the whole accumulation shares ONE
    # scale (sU*sx) applied to u on the host.  This halves x's HBM bytes
    # where the cast engines still have capacity; full-x-int8 turns the
    # cast stream into the wall.
    XQ = NCH // 2
    nc = bacc.Bacc("TRN2", target_bir_lowering=False, debug=False,
                   num_devices=N_CORES)
    uh = nc.dram_tensor("uh", [I_PER_CORE, 128, NCH, CO], mybir.dt.int8,
                        kind="ExternalInput").ap()
    xhq = nc.dram_tensor("xhq", [I_PER_CORE, 128, XQ, B], mybir.dt.int8,
                         kind="ExternalInput").ap()
    xhb = nc.dram_tensor("xhb", [I_PER_CORE, 128, NCH - XQ, B], BF16,
                         kind="ExternalInput").ap()
    u_out = nc.dram_tensor("u_out", [CO, I_PER_CORE, B], BF16,
                           kind="ExternalOutput").ap()

    with tile.TileContext(nc) as tc, ExitStack() as ctx:
        qp = ctx.enter_context(tc.tile_pool(name="uq", bufs=3))
        up = ctx.enter_context(tc.tile_pool(name="ut", bufs=3))
        xqp = ctx.enter_context(tc.tile_pool(name="xq", bufs=3))
        xp = ctx.enter_context(tc.tile_pool(name="xt", bufs=3))
        pp = ctx.enter_context(
            tc.tile_pool(name="ps", bufs=2, space=bass.MemorySpace.PSUM))
        sp = ctx.enter_context(tc.tile_pool(name="usb", bufs=1))

        H = NCH // 2
        for _ in range(repeat):
            u_sb = sp.tile([CO, I_PER_CORE, B], BF16)
            for i in range(I_PER_CORE):
                uq = qp.tile([128, NCH, CO], mybir.dt.int8)
                nc.sync.dma_start(uq[:, :H, :], uh[i, :, :H, :])
                nc.scalar.dma_start(uq[:, H:, :], uh[i, :, H:, :])
                xq = xqp.tile([128, XQ, B], mybir.dt.int8)
                xt = xp.tile([128, NCH, B], BF16)
                # alternate x loads between the rings to keep them balanced
                xeng = nc.sync if i % 2 == 0 else nc.scalar
                xeng2 = nc.scalar if i % 2 == 0 else nc.sync
                xeng.dma_start(xq[:], xhq[i])
                xeng2.dma_start(xt[:, XQ:, :], xhb[i])
                ut = up.tile([128, NCH, CO], BF16)
                nc.vector.tensor_copy(ut[:, :dve_chunks, :],
                                      uq[:, :dve_chunks, :])
                nc.gpsimd.tensor_copy(ut[:, dve_chunks:, :],
                                      uq[:, dve_chunks:, :])
                # x int8 half: cast split DVE / Pool (Act stays DMA-only —
                # its activation-table load alone costs 1.3us).  A last-tile
                # fine-grained cast split (sim -375ns) was tried but the HW
                # run hit NRT_EXEC_UNIT_UNRECOVERABLE — reverted to this
                # HW-validated pattern.
                nc.vector.tensor_copy(xt[:, :10, :], xq[:, :10, :])
                nc.gpsimd.tensor_copy(xt[:, 10:XQ, :], xq[:, 10:, :])
                ps = pp.tile([CO, B], F32)
                for c in range(NCH):
                    nc.tensor.matmul(ps[:], ut[:, c, :], xt[:, c, :],
                                     start=(c == 0), stop=(c == NCH - 1))
                nc.vector.tensor_copy(u_sb[:, i, :], ps[:])
            # split the result store: i=0..6 go out early on Pool, the final
            # 128B/partition sliver rides the sync ring (idle by then) with
            # its cheaper 1,717ns sem delay, shortening the kernel tail
            nc.gpsimd.dma_start(u_out[:, :I_PER_CORE - 1, :],
                                u_sb[:, :I_PER_CORE - 1, :])
            nc.sync.dma_start(u_out[:, I_PER_CORE - 1, :],
                              u_sb[:, I_PER_CORE - 1, :])
    nc.compile()
    return nc


NPAIR = O_PER_CORE // 2  # 8 o-pairs per core


def _build_stage_b(repeat=1, vt_bufs=8, zsb_bufs=8, ps_bufs=4,
                   act_copies=(3, 5, 7, 9, 11, 13, 15),
                   pool_stores=(0, 1, 2, 3, 4, 6),
                   sp_loads=(0, 2, 4, 5, 6), store_map=None,
                   lhalf=2):
    # K=64 contraction (biases are added on the host, where V3.sum(-1) is
    # already computed), which lets TWO o-blocks pack vertically into the
    # 128 SBUF partitions: vh[p] = [128, S_OUT] with rows 0..63 = V block
    # of o=2p and rows 64..127 = o=2p+1.  Every DMA then runs at the full
    # 128-partition rate, halving the modeled V-load time vs a 66-row
    # layout.  us arrives host-transposed [64, 16, B].
    nc = bacc.Bacc("TRN2", target_bir_lowering=False, debug=False,
                   num_devices=N_CORES)
    vh = nc.dram_tensor("vh", [NPAIR, 128, S_OUT], BF16,
                        kind="ExternalInput").ap()
    us = nc.dram_tensor("us", [CI, O_PER_CORE, B], BF16,
                        kind="ExternalInput").ap()
    z_out = nc.dram_tensor("z_out", [NPAIR, 128, 2, TT, B], BF16,
                           kind="ExternalOutput").ap()

    with tile.TileContext(nc) as tc, ExitStack() as ctx:
        # V loads stay on the two HWDGE rings only.  Balancing them onto the
        # SWDGE path is faster in the cost model but crashed the device
        # (NRT_EXEC_UNIT_UNRECOVERABLE) intermittently on real HW in both
        # pool configurations tried -- not safe for a one-shot run.
        vp = ctx.enter_context(tc.tile_pool(name="vt", bufs=vt_bufs))
        usp = ctx.enter_context(tc.tile_pool(name="ust", bufs=1))
        pp = ctx.enter_context(
            tc.tile_pool(name="ps", bufs=ps_bufs, space=bass.MemorySpace.PSUM))
        zp = ctx.enter_context(tc.tile_pool(name="zsb", bufs=zsb_bufs))

        for _ in range(repeat):
            # us loaded once into partitions 0..63, then duplicated on-chip
            # to 64..127 (SBUF->SBUF DMA, no HBM bytes) so both pair halves
            # have a rhs at the same base partition as their lhsT
            us_all = usp.tile([2 * CI, O_PER_CORE, B], BF16)
            nc.gpsimd.dma_start(us_all[:CI], us[:])
            nc.gpsimd.dma_start(us_all[CI:], us_all[:CI])
            # hoist ALL V loads ahead of every store in program order so the
            # Tile scheduler never lets a store block a later load on a ring
            vts = []
            for p in range(NPAIR):
                # SP takes 5 pair loads, Act 3: Act's queue starts ~1.3us
                # late (activation-table load for its copies), so the two
                # rings then finish their loads at about the same time
                le = nc.sync if p in sp_loads else nc.scalar
                vt = vp.tile([128, S_OUT], BF16)
                if p == 0:
                    # split the first load into column halves so the first
                    # matmuls (and the copy stream they feed) start earlier
                    le.dma_start(vt[:, :S_OUT // 2], vh[p][:, :S_OUT // 2])
                    le.dma_start(vt[:, S_OUT // 2:], vh[p][:, S_OUT // 2:])
                else:
                    le.dma_start(vt[:], vh[p])
                vts.append(vt)
            for p in range(NPAIR):
                # most z stores ride the Pool SWDGE queue (SBUF->DRAM via
                # gpsimd — the same proven pattern as stage A's u_out store)
                # to keep both HWDGE rings free for loads and copies
                if store_map is not None:
                    se = store_map(nc, p)
                elif p in pool_stores:
                    se = nc.gpsimd
                else:
                    se = nc.scalar if p % 2 == 0 else nc.sync
                vt = vts[p]
                z_sb = zp.tile([128, 2, TT, B], BF16)
                for half in range(lhalf):
                    j = p * 2 + half
                    lhs = vt[half * CI:(half + 1) * CI, :]
                    rhs_us = us_all[half * CI:(half + 1) * CI]
                    ps = pp.tile([128, TT, B], F32)
                    for tt in range(TT):
                        nc.tensor.matmul(ps[:, tt, :],
                                         lhs[:, bass.ts(tt, 128)],
                                         rhs_us[:, j, :],
                                         start=True, stop=True)
                    # PSUM->SBUF casts: only DVE and Act may read PSUM
                    # (the BIR verifier rejects GPSIMD-PSUM access)
                    if j in act_copies:
                        nc.scalar.copy(z_sb[:, half, :, :], ps[:])
                    else:
                        nc.vector.tensor_copy(z_sb[:, half, :, :], ps[:])
                    if p == NPAIR - 1:
                        # store the final pair per half on SEPARATE rings so
                        # neither last store waits for the other copy or
                        # serializes behind its sibling
                        e2 = nc.sync if half == 0 else nc.scalar
                        e2.dma_start(z_out[p][:, half], z_sb[:, half])
                if p != NPAIR - 1:
                    se.dma_start(z_out[p], z_sb[:])
    nc.compile()
    return nc


def _get(name):
    if name not in _cache:
        _cache[name] = _build_stage_a() if name == "a" else _build_stage_b()
    return _cache[name]


def _run(nc, in_maps, attempts=3):
    last = None
    for k in range(attempts):
        try:
            return run_bass_kernel_spmd(nc, in_maps, CORE_IDS).results
        except Exception as e:     # transient axon/runtime hiccups
            last = e
            if k + 1 < attempts:
                import time as _t
                _t.sleep(15 * (k + 1))
    raise last


def kernel(x, U, bU, V, bV):
    x = np.asarray(x, np.float32)
    U = np.asarray(U, np.float32)
    bU = np.asarray(bU, np.float32)
    V = np.asarray(V, np.float32)
    bV = np.asarray(bV, np.float32)

    # ---- host prep: contiguous-DMA layouts ----
    # U -> symmetric int8 (uniform distribution, ~0.4% RMS).  x is hybrid:
    # chunks 0..15 int8 (4-sigma clip, ~0.9% RMS on half the contraction),
    # chunks 16..31 bf16 pre-scaled by 1/sx, so the device accumulates a
    # single consistent scale and u_true = u_raw * (sU*sx), applied on the
    # host below.  No scaling ops exist on device.
    sU = np.abs(U).max() / 127.0
    sx = 4.0 / 127.0
    Uq = np.clip(np.rint(U / sU), -127, 127).astype(np.int8)
    Uh = Uq.reshape(CO, CI, NCH, 128).transpose(1, 3, 2, 0)
    X4 = x.reshape(B, CI, NCH, 128).transpose(1, 3, 2, 0)  # [i, s128, c, b]
    XQh = np.clip(np.rint(X4[:, :, :NCH // 2, :] / sx),
                  -127, 127).astype(np.int8)
    XBh = (X4[:, :, NCH // 2:, :] / sx).astype(NP_BF16)

    in_maps_a = []
    for c in range(N_CORES):
        sl = slice(c * I_PER_CORE, (c + 1) * I_PER_CORE)
        in_maps_a.append({
            "uh": np.ascontiguousarray(Uh[sl]),
            "xhq": np.ascontiguousarray(XQh[sl]),
            "xhb": np.ascontiguousarray(XBh[sl]),
        })

    nc_a = _get("a")
    res_a = _run(nc_a, in_maps_a)
    # u_all[o, i, b]: raw sums at scale 1/(sU*sx); fold the scale back in
    u_all = np.concatenate([res_a[c]["u_out"] for c in range(N_CORES)], axis=1)
    u_all = (u_all.astype(np.float32) * (sU * sx)).astype(NP_BF16)

    # ---- host mid: o-pair-packed V, bias table stays on the host ----
    V3 = V.reshape(S_OUT, CO, CI)
    # Vper[o, i, t]; pairs pack as [CO//2, 128, S_OUT] (rows 0..63 = even o)
    Vper = np.ascontiguousarray(V3.transpose(1, 2, 0)).astype(NP_BF16)
    Vh = Vper.reshape(CO // 2, 2 * CI, S_OUT)
    # bias[o, t] = bU[o] * sum_i V3[t, o, i] + bV[t]  (added in fp32 at the end)
    bias = bU[:, None] * V3.sum(-1).T + bV[None, :]

    in_maps_b = []
    for c in range(N_CORES):
        sl = slice(c * O_PER_CORE, (c + 1) * O_PER_CORE)
        in_maps_b.append({
            "vh": np.ascontiguousarray(Vh[c * NPAIR:(c + 1) * NPAIR]),
            "us": np.ascontiguousarray(u_all[sl].transpose(1, 0, 2)),
        })

    nc_b = _get("b")
    res_b = _run(nc_b, in_maps_b)
    # per-core z_out: [NPAIR, t128, 2, tt, b] -> [o_local, t128, tt, b]
    z_all = np.concatenate(
        [res_b[c]["z_out"].transpose(0, 2, 1, 3, 4).reshape(
            O_PER_CORE, 128, TT, B) for c in range(N_CORES)], axis=0)

    # ---- host final: z[b, o, t] with t = tt*128 + t128, plus bias ----
    z = z_all.astype(np.float32).transpose(3, 0, 2, 1).reshape(B, CO, S_OUT)
    z = z + bias[None, :, :]
    return np.ascontiguousarray(z.reshape(B, CO, 32, 32))


# revision 43
# speedup vs baseline: 1.0135x; 1.0135x over previous
"""Trainium2 Bass kernel for nn_FLinear2d (per-channel double linear).

Math (see reference):
  u[b,i,o] = sum_s U3[o,i,s] * x[b,i,s] + bU[o]        (64 per-channel matmuls)
  z[b,o,t] = sum_i V3[t,o,i] * u[b,i,o] + bV[t]        (128 per-o matmuls)

Two SPMD launches over 8 cores, all matmul operands in bf16 (fp32 PSUM
accumulation).  The problem is HBM-bound — U alone is 134 MB fp32 — and the
2e-2 gate leaves ~10x margin over bf16's ~3e-3 error (K=4096 accumulation in
fp32), so bf16 halves the dominant traffic and quadruples PE throughput.

  Stage A: shard C_in (8 channels/core).  Per (i, s-chunk):
      psum[o=128, b=64] += Uh[i,:,c,:].T @ Xh[i,:,c,:]   (bf16 in, fp32 acc)
    accumulated over 32 s-chunks -> u_base[o, i, b] (bf16 out).
  Stage B: shard C_out (16 o/core), pure K=64 contraction (biases are a
  host-side rank-1 table: bias[o,t] = bU[o]*sum_i V3[t,o,i] + bV[t], added
  in fp32 after unsharding).  V arrives o-PAIR packed: vh[p] = [128, S_OUT]
  with rows 0..63 = o=2p's V block and 64..127 = o=2p+1's, so every DMA
  uses all 128 SBUF partitions.  us is loaded once and duplicated on-chip
  (SBUF->SBUF) so both halves have a matching base partition.
    Per (o, t-tile): psum[t=128, b=64] = vt[half][:, tt*128:+128].T @ us[o]
    (single matmul, K=64), stored to HBM as bf16 pair blocks.

All DMAs are contiguous thanks to host-side layout transforms.
"""

import numpy as np
import ml_dtypes
from contextlib import ExitStack

import concourse.bass as bass
import concourse.tile as tile
from concourse import bacc, mybir
from concourse.bass_utils import run_bass_kernel_spmd

F32 = mybir.dt.float32
BF16 = mybir.dt.bfloat16
NP_BF16 = ml_dtypes.bfloat16
N_CORES = 8
CORE_IDS = list(range(N_CORES))

B, CI, CO = 64, 64, 128
S_IN, S_OUT = 4096, 1024
NCH = 32            # s-chunks of 128
I_PER_CORE = CI // N_CORES     # 8
O_PER_CORE = CO // N_CORES     # 16
TT = S_OUT // 128   # 8 t-tiles per o

_cache = {}


def _build_stage_a(repeat=1, dve_chunks=18):
    # U streams from HBM as int8 (it is uniformly distributed, so symmetric
    # int8 has ~0.4% RMS error vs fp8's 3.6% — measured end-to-end 5.5e-3
    # against the 2e-2 gate).  The dequant is a pure int8->bf16 cast
    # (integers <=127 are exact in bf16); the quant step is folded into x
    # on the host, so no extra scaling op exists anywhere on device.
    # Casts split DVE (0.58 ns/elem) / Pool (0.83); both rings carry
    # (U/2 + x/2) at ~12.6 us, all under the 23.8 us HBM floor.
    # (x-int8 as well was tried: floor 17.9us but the cast stream becomes
    # the wall at ~23.4us sim and error rises to 1.2e-2 — not worth it.)
    # HYBRID x: chunks 0..15 stream as int8 (cast on-chip), 16..31 as bf16
    # pre-scaled by 1/sx on the host, so the whole accumulation shares ONE
    # scale (sU*sx) applied to u on the host.  This halves x's HBM bytes
    # where the cast engines still have capacity; full-x-int8 turns the
    # cast stream into the wall.
    XQ = NCH // 2
    nc = bacc.Bacc("TRN2", target_bir_lowering=False, debug=False,
                   num_devices=N_CORES)
    uh = nc.dram_tensor("uh", [I_PER_CORE, 128, NCH, CO], mybir.dt.int8,
                        kind="ExternalInput").ap()
    xhq = nc.dram_tensor("xhq", [I_PER_CORE, 128, XQ, B], mybir.dt.int8,
                         kind="ExternalInput").ap()
    xhb = nc.dram_tensor("xhb", [I_PER_CORE, 128, NCH - XQ, B], BF16,
                         kind="ExternalInput").ap()
    u_out = nc.dram_tensor("u_out", [CO, I_PER_CORE, B], BF16,
                           kind="ExternalOutput").ap()

    with tile.TileContext(nc) as tc, ExitStack() as ctx:
        qp = ctx.enter_context(tc.tile_pool(name="uq", bufs=3))
        up = ctx.enter_context(tc.tile_pool(name="ut", bufs=3))
        xqp = ctx.enter_context(tc.tile_pool(name="xq", bufs=3))
        xp = ctx.enter_context(tc.tile_pool(name="xt", bufs=3))
        pp = ctx.enter_context(
            tc.tile_pool(name="ps", bufs=2, space=bass.MemorySpace.PSUM))
        sp = ctx.enter_context(tc.tile_pool(name="usb", bufs=1))

        H = NCH // 2
        for _ in range(repeat):
            u_sb = sp.tile([CO, I_PER_CORE, B], BF16)
            for i in range(I_PER_CORE):
                uq = qp.tile([128, NCH, CO], mybir.dt.int8)
                nc.sync.dma_start(uq[:, :H, :], uh[i, :, :H, :])
                nc.scalar.dma_start(uq[:, H:, :], uh[i, :, H:, :])
                xq = xqp.tile([128, XQ, B], mybir.dt.int8)
                xt = xp.tile([128, NCH, B], BF16)
                # alternate x loads between the rings to keep them balanced
                xeng = nc.sync if i % 2 == 0 else nc.scalar
                xeng2 = nc.scalar if i % 2 == 0 else nc.sync
                xeng.dma_start(xq[:], xhq[i])
                xeng2.dma_start(xt[:, XQ:, :], xhb[i])
                ut = up.tile([128, NCH, CO], BF16)
                nc.vector.tensor_copy(ut[:, :dve_chunks, :],
                                      uq[:, :dve_chunks, :])
                nc.gpsimd.tensor_copy(ut[:, dve_chunks:, :],
                                      uq[:, dve_chunks:, :])
                # x int8 half: cast split DVE / Pool (Act stays DMA-only —
                # its activation-table load alone costs 1.3us).  A last-tile
                # fine-grained cast split (sim -375ns) was tried but the HW
                # run hit NRT_EXEC_UNIT_UNRECOVERABLE — reverted to this
                # HW-validated pattern.
                nc.vector.tensor_copy(xt[:, :10, :], xq[:, :10, :])
                nc.gpsimd.tensor_copy(xt[:, 10:XQ, :], xq[:, 10:, :])
                ps = pp.tile([CO, B], F32)
                # accumulate bf16-x chunks (16..31) FIRST: they depend only
                # on the U casts, so the matmul stream starts before the
                # x casts land and ends sooner after the last cast (PSUM
                # accumulation is order-independent)
                order = list(range(XQ, NCH)) + list(range(XQ))
                for k, c in enumerate(order):
                    nc.tensor.matmul(ps[:], ut[:, c, :], xt[:, c, :],
                                     start=(k == 0), stop=(k == NCH - 1))
                nc.vector.tensor_copy(u_sb[:, i, :], ps[:])
            # split the result store: i=0..6 go out early on Pool, the final
            # 128B/partition sliver rides the sync ring (idle by then) with
            # its cheaper 1,717ns sem delay, shortening the kernel tail
            nc.gpsimd.dma_start(u_out[:, :I_PER_CORE - 1, :],
                                u_sb[:, :I_PER_CORE - 1, :])
            nc.sync.dma_start(u_out[:, I_PER_CORE - 1, :],
                              u_sb[:, I_PER_CORE - 1, :])
    nc.compile()
    return nc


NPAIR = O_PER_CORE // 2  # 8 o-pairs per core


def _build_stage_b(repeat=1, vt_bufs=8, zsb_bufs=8, ps_bufs=4,
                   act_copies=(3, 5, 7, 9, 11, 13, 15),
                   pool_stores=(0, 1, 2, 3, 4, 6),
                   sp_loads=(0, 2, 4, 5, 6), store_map=None,
                   lhalf=2):
    # K=64 contraction (biases are added on the host, where V3.sum(-1) is
    # already computed), which lets TWO o-blocks pack vertically into the
    # 128 SBUF partitions: vh[p] = [128, S_OUT] with rows 0..63 = V block
    # of o=2p and rows 64..127 = o=2p+1.  Every DMA then runs at the full
    # 128-partition rate, halving the modeled V-load time vs a 66-row
    # layout.  us arrives host-transposed [64, 16, B].
    nc = bacc.Bacc("TRN2", target_bir_lowering=False, debug=False,
                   num_devices=N_CORES)
    vh = nc.dram_tensor("vh", [NPAIR, 128, S_OUT], BF16,
                        kind="ExternalInput").ap()
    us = nc.dram_tensor("us", [CI, O_PER_CORE, B], BF16,
                        kind="ExternalInput").ap()
    z_out = nc.dram_tensor("z_out", [NPAIR, 128, 2, TT, B], BF16,
                           kind="ExternalOutput").ap()

    with tile.TileContext(nc) as tc, ExitStack() as ctx:
        # V loads stay on the two HWDGE rings only.  Balancing them onto the
        # SWDGE path is faster in the cost model but crashed the device
        # (NRT_EXEC_UNIT_UNRECOVERABLE) intermittently on real HW in both
        # pool configurations tried -- not safe for a one-shot run.
        vp = ctx.enter_context(tc.tile_pool(name="vt", bufs=vt_bufs))
        usp = ctx.enter_context(tc.tile_pool(name="ust", bufs=1))
        pp = ctx.enter_context(
            tc.tile_pool(name="ps", bufs=ps_bufs, space=bass.MemorySpace.PSUM))
        zp = ctx.enter_context(tc.tile_pool(name="zsb", bufs=zsb_bufs))

        for _ in range(repeat):
            # us loaded once into partitions 0..63, then duplicated on-chip
            # to 64..127 (SBUF->SBUF DMA, no HBM bytes) so both pair halves
            # have a rhs at the same base partition as their lhsT
            us_all = usp.tile([2 * CI, O_PER_CORE, B], BF16)
            nc.gpsimd.dma_start(us_all[:CI], us[:])
            nc.gpsimd.dma_start(us_all[CI:], us_all[:CI])
            # hoist ALL V loads ahead of every store in program order so the
            # Tile scheduler never lets a store block a later load on a ring
            vts = []
            for p in range(NPAIR):
                # SP takes 5 pair loads, Act 3: Act's queue starts ~1.3us
                # late (activation-table load for its copies), so the two
                # rings then finish their loads at about the same time
                le = nc.sync if p in sp_loads else nc.scalar
                vt = vp.tile([128, S_OUT], BF16)
                if p == 0:
                    # split the first load into column halves so the first
                    # matmuls (and the copy stream they feed) start earlier
                    le.dma_start(vt[:, :S_OUT // 2], vh[p][:, :S_OUT // 2])
                    le.dma_start(vt[:, S_OUT // 2:], vh[p][:, S_OUT // 2:])
                else:
                    le.dma_start(vt[:], vh[p])
                vts.append(vt)
            for p in range(NPAIR):
                # most z stores ride the Pool SWDGE queue (SBUF->DRAM via
                # gpsimd — the same proven pattern as stage A's u_out store)
                # to keep both HWDGE rings free for loads and copies
                if store_map is not None:
                    se = store_map(nc, p)
                elif p in pool_stores:
                    se = nc.gpsimd
                else:
                    se = nc.scalar if p % 2 == 0 else nc.sync
                vt = vts[p]
                z_sb = zp.tile([128, 2, TT, B], BF16)
                for half in range(lhalf):
                    j = p * 2 + half
                    lhs = vt[half * CI:(half + 1) * CI, :]
                    rhs_us = us_all[half * CI:(half + 1) * CI]
                    ps = pp.tile([128, TT, B], F32)
                    for tt in range(TT):
                        nc.tensor.matmul(ps[:, tt, :],
                                         lhs[:, bass.ts(tt, 128)],
                                         rhs_us[:, j, :],
                                         start=True, stop=True)
                    # PSUM->SBUF casts: only DVE and Act may read PSUM
                    # (the BIR verifier rejects GPSIMD-PSUM access)
                    if j in act_copies:
                        nc.scalar.copy(z_sb[:, half, :, :], ps[:])
                    else:
                        nc.vector.tensor_copy(z_sb[:, half, :, :], ps[:])
                    if p == NPAIR - 1:
                        # store the final pair per half on SEPARATE rings so
                        # neither last store waits for the other copy or
                        # serializes behind its sibling
                        e2 = nc.sync if half == 0 else nc.scalar
                        e2.dma_start(z_out[p][:, half], z_sb[:, half])
                if p != NPAIR - 1:
                    se.dma_start(z_out[p], z_sb[:])
    nc.compile()
    return nc


def _get(name):
    if name not in _cache:
        _cache[name] = _build_stage_a() if name == "a" else _build_stage_b()
    return _cache[name]


def _run(nc, in_maps, attempts=3):
    last = None
    for k in range(attempts):
        try:
            return run_bass_kernel_spmd(nc, in_maps, CORE_IDS).results
        except Exception as e:     # transient axon/runtime hiccups
            last = e
            if k + 1 < attempts:
                import time as _t
                _t.sleep(15 * (k + 1))
    raise last


def kernel(x, U, bU, V, bV):
    x = np.asarray(x, np.float32)
    U = np.asarray(U, np.float32)
    bU = np.asarray(bU, np.float32)
    V = np.asarray(V, np.float32)
    bV = np.asarray(bV, np.float32)

    # ---- host prep: contiguous-DMA layouts ----
    # U -> symmetric int8 (uniform distribution, ~0.4% RMS).  x is hybrid:
    # chunks 0..15 int8 (4-sigma clip, ~0.9% RMS on half the contraction),
    # chunks 16..31 bf16 pre-scaled by 1/sx, so the device accumulates a
    # single consistent scale and u_true = u_raw * (sU*sx), applied on the
    # host below.  No scaling ops exist on device.
    sU = np.abs(U).max() / 127.0
    sx = 4.0 / 127.0
    Uq = np.clip(np.rint(U / sU), -127, 127).astype(np.int8)
    Uh = Uq.reshape(CO, CI, NCH, 128).transpose(1, 3, 2, 0)
    X4 = x.reshape(B, CI, NCH, 128).transpose(1, 3, 2, 0)  # [i, s128, c, b]
    XQh = np.clip(np.rint(X4[:, :, :NCH // 2, :] / sx),
                  -127, 127).astype(np.int8)
    XBh = (X4[:, :, NCH // 2:, :] / sx).astype(NP_BF16)

    in_maps_a = []
    for c in range(N_CORES):
        sl = slice(c * I_PER_CORE, (c + 1) * I_PER_CORE)
        in_maps_a.append({
            "uh": np.ascontiguousarray(Uh[sl]),
            "xhq": np.ascontiguousarray(XQh[sl]),
            "xhb": np.ascontiguousarray(XBh[sl]),
        })

    nc_a = _get("a")
    res_a = _run(nc_a, in_maps_a)
    # u_all[o, i, b]: raw sums at scale 1/(sU*sx); fold the scale back in
    u_all = np.concatenate([res_a[c]["u_out"] for c in range(N_CORES)], axis=1)
    u_all = (u_all.astype(np.float32) * (sU * sx)).astype(NP_BF16)

    # ---- host mid: o-pair-packed V, bias table stays on the host ----
    V3 = V.reshape(S_OUT, CO, CI)
    # Vper[o, i, t]; pairs pack as [CO//2, 128, S_OUT] (rows 0..63 = even o)
    Vper = np.ascontiguousarray(V3.transpose(1, 2, 0)).astype(NP_BF16)
    Vh = Vper.reshape(CO // 2, 2 * CI, S_OUT)
    # bias[o, t] = bU[o] * sum_i V3[t, o, i] + bV[t]  (added in fp32 at the end)
    bias = bU[:, None] * V3.sum(-1).T + bV[None, :]

    in_maps_b = []
    for c in range(N_CORES):
        sl = slice(c * O_PER_CORE, (c + 1) * O_PER_CORE)
        in_maps_b.append({
            "vh": np.ascontiguousarray(Vh[c * NPAIR:(c + 1) * NPAIR]),
            "us": np.ascontiguousarray(u_all[sl].transpose(1, 0, 2)),
        })

    nc_b = _get("b")
    res_b = _run(nc_b, in_maps_b)
    # per-core z_out: [NPAIR, t128, 2, tt, b] -> [o_local, t128, tt, b]
    z_all = np.concatenate(
        [res_b[c]["z_out"].transpose(0, 2, 1, 3, 4).reshape(
            O_PER_CORE, 128, TT, B) for c in range(N_CORES)], axis=0)

    # ---- host final: z[b, o, t] with t = tt*128 + t128, plus bias ----
    z = z_all.astype(np.float32).transpose(3, 0, 2, 1).reshape(B, CO, S_OUT)
    z = z + bias[None, :, :]
    return np.ascontiguousarray(z.reshape(B, CO, 32, 32))
